# revision 21
# baseline (speedup 1.0000x reference)
"""Trainium2 Bass kernel for nn_BiDirectionalMultiHeadPointer.

Math notes (derived from the reference, verified numerically):

The reference broadcasts a *scalar* per-(h,b,n) logit across the softmax
axis, so every softmax row is constant and the probabilities are exactly
uniform 1/N.  Consequences:

  * fwd/bwd encoder MLPs only feed those logits -> completely dead code.
  * attention-weighted features == mean over n of head_features, which is
    (mean_n h) @ Wv^T -- a rank-1-per-batch bias term after the fusion
    layer-1 weight is applied.
  * pointer targets before chaining are sum((1/N)*pos) = 511.5 -> int32
    (round-to-nearest on the device backend the reference runs under: 512).
  * entropy "strength" is the constant 1 - log(1/N + 1e-8).

With the per-head fusion weights folded into the value projection and the
output projection (exact linear algebra, done once on the host in f64):

  C1[j,d]   = sum_c fu_W1[h,e,:HD][e,c] * Wv[h,c,d]      (j = h*HD+e)
  b1[b,j]   = fu_b1[h,e] + (fu_W1[:,HD:2HD]+fu_W1[:,2HD:])[h,e,:] @ (Wv[h] @ mean_n h[b])
  G[j,e']   = sum_f Wo[e', h*HD+f] * fu_W2[h,f,e]        (j = h*HD+e)
  b_out     = Wo @ vec(fu_b2)

  final[b,n] = gelu(h[b,n] @ C1^T + b1[b]) @ G + b_out

The device kernel computes exactly that: two [256,1024]x[1024,1024]
matmuls with an erf-GELU between them, token-sharded 256 rows/core over
8 cores (a core's rows all belong to one batch, so b1 is a per-core
per-partition bias folded into the activation instruction).

Performance structure (from trace analysis):
  * matmul operands are fp16: the PE streams 2-byte dtypes at 1 cyc/row
    (fp32 needs 4, fp32r/TF32 2), fp16 carries the same 10-bit mantissa
    as the PE's TF32 mode, and DMA bytes halve. The tiny folded weights
    are pre-scaled by a power of two into fp16's normal range (exact);
    GELU's scale argument / a per-partition tensor_scalar undo it.
  * all DRAM operands are pre-tiled on the host so every DMA descriptor
    is a contiguous multi-KB run.
  * all large loads share the SP HWDGE ring, FIFO-ordered by consumption
    time (w0,w1,w2,g0,w3,g1,...); activations ride the gpsimd SWDGE
    queue in quarters so the first matmul starts ~4us earlier.
  * the second matmul trails the first by one j-block (software
    pipeline), so the PE never stalls on a GELU; accumulation groups for
    the 4 output PSUM banks stay open across the whole j loop.
  * a short burst of dense dummy matmuls on a memset tile warms the PE
    HAM clock gate (cold PE runs at 1.2 GHz for ~3.4us) while the first
    real operands are still in flight.
"""

import numpy as np

import concourse.mybir as mybir
import concourse.tile as tile
from concourse import bacc
from concourse import bass_utils

B, N, D, H, HD = 2, 1024, 1024, 16, 64
NCORES = 8
ROWS = (B * N) // NCORES      # 256 rows per core
DT = mybir.dt.float32
MM_DT = mybir.dt.float16      # matmul operand dtype: fp16 streams 1 cyc/row
# fp16 has the same 10-bit mantissa as the PE's fp32r/TF32 mode but half the
# bytes; the small-magnitude folded weights are pre-scaled by a power of two
# into fp16's normal range (exact), and the scale is undone on-chip (GELU's
# scale argument / a per-partition tensor_scalar multiply).

_CACHE = {}


def _build_nc(has_bout):
    """Build + compile the SPMD single-core program (same on all 8 cores)."""
    nc = bacc.Bacc(
        "TRN2",
        target_bir_lowering=False,
        debug=False,
        enable_asserts=False,
        num_devices=NCORES,
    )

    def mm(out, lhsT, rhs, start, stop):
        nc.tensor.matmul(out, lhsT, rhs, start=start, stop=stop)

    # host-pre-tiled so each [128, X] load is 128 contiguous 4-8KB runs
    hT = nc.dram_tensor("ht", [128, 8, ROWS], MM_DT, kind="ExternalInput").ap()
    c1tw = nc.dram_tensor("c1tw", [8, 128, 8, 128], MM_DT, kind="ExternalInput").ap()
    g = nc.dram_tensor("g", [D, D], MM_DT, kind="ExternalInput").ap()
    b1 = nc.dram_tensor("b1", [128, 8], DT, kind="ExternalInput").ap()
    scales = nc.dram_tensor("scales", [128, 2], DT, kind="ExternalInput").ap()
    bout = (nc.dram_tensor("bout", [1, D], DT, kind="ExternalInput").ap()
            if has_bout else None)
    out = nc.dram_tensor("out", [ROWS, D], DT, kind="ExternalOutput").ap()

    with tile.TileContext(nc) as tc:
        with (
            tc.tile_pool(name="hpool", bufs=1) as hpool,
            tc.tile_pool(name="wpool", bufs=6) as wpool,
            tc.tile_pool(name="gpool", bufs=8) as gpool,
            tc.tile_pool(name="xpool", bufs=8) as xpool,
            tc.tile_pool(name="bpool", bufs=1) as bpool,
            tc.tile_pool(name="opool", bufs=4) as opool,
            tc.tile_pool(name="mm1p", bufs=2, space="PSUM") as mm1p,
            tc.tile_pool(name="mm2p", bufs=1, space="PSUM") as mm2p,
        ):
            # --- resident inputs ----------------------------------------------
            # ht in quarters on the gpsimd (SWDGE) queue so the first
            # matmul isn't queued behind it on the weight ring
            ht_q = []
            b1_t = None
            for hh in range(4):
                ht_t = hpool.tile([128, 2, ROWS], MM_DT, name=f"ht{hh}",
                                  tag=f"ht{hh}")   # [p, dc, r], d = (2*hh+dc)*128+p
                nc.gpsimd.dma_start(out=ht_t, in_=hT[:, 2 * hh:2 * (hh + 1), :])
                ht_q.append(ht_t)
                if hh == 0:
                    b1_t = bpool.tile([128, 8], DT)        # j = jc*128 + p
                    nc.gpsimd.dma_start(out=b1_t, in_=b1)
                    sc_t = bpool.tile([128, 2], DT)        # [1/S1, 1/S2]
                    nc.gpsimd.dma_start(out=sc_t, in_=scales)

            if has_bout:
                bout_t = bpool.tile([128, D], DT)
                nc.gpsimd.dma_start(out=bout_t, in_=bout.broadcast_to((128, D)))

            out_ps = [mm2p.tile([128, 512], DT, name=f"outp{i}", tag=f"outp{i}")
                      for i in range(4)]

            # HAM warm-up: dense N=512 matmuls on a memset tile keep the PE
            # busy-window >90% occupied until the first real matmul, so the
            # clock gate opens (cold PE = 1.2 GHz) before real work arrives
            one_c = nc.const_aps.aps[(mybir.dt.float32, 1.0)]
            wsrc = bpool.tile([128, 512], MM_DT, name="wsrc", tag="wsrc")
            nc.vector.memset(wsrc, 0.0)
            one16 = bpool.tile([128, 1], MM_DT, name="one16", tag="one16")
            nc.vector.tensor_copy(one16, one_c)
            warm = mm1p.tile([1, 512], DT, name="warm", tag="warm")
            for i in range(8):
                nc.tensor.matmul(warm, one16, wsrc,
                                 start=(i == 0), stop=(i == 7))

            # ---- all large loads on the sync HWDGE ring, FIFO-ordered by
            # consumption time: w blocks pace mm1; g[jc] is needed one jc
            # later by mm2, so it rides two slots behind its w ------------
            w_tiles, g_tiles = {}, {}

            def load_w(jc):
                if jc < 2:
                    w_ab = []
                    for hh in range(2):
                        w_t = wpool.tile([128, 4, 128], MM_DT, name=f"w{hh}",
                                         tag=f"w{hh}")
                        nc.sync.dma_start(
                            out=w_t, in_=c1tw[jc, :, 4 * hh:4 * (hh + 1), :])
                        w_ab.append(w_t)
                    w_tiles[jc] = w_ab
                else:
                    w_t = wpool.tile([128, 8, 128], MM_DT, name="wf", tag="wf")
                    nc.sync.dma_start(out=w_t, in_=c1tw[jc])
                    w_tiles[jc] = [w_t[:, :4, :], w_t[:, 4:, :]]

            def load_g(jc):
                g_t = gpool.tile([128, D], MM_DT, name="gt", tag="gt")
                nc.sync.dma_start(out=g_t, in_=g[jc * 128:(jc + 1) * 128, :])
                g_tiles[jc] = g_t

            for item in ("w0", "w1", "w2", "g0", "w3", "g1", "w4", "g2",
                         "w5", "g3", "w6", "g4", "w7", "g5", "g6", "g7"):
                (load_w if item[0] == "w" else load_g)(int(item[1]))

            def mm2(jc, xg):
                # out[rh, ec] += xg^T[rh] @ G[jc, ec]
                g_t = g_tiles[jc]
                for rh in range(2):
                    for ec in range(2):
                        mm(out_ps[rh * 2 + ec],
                           xg[:, rh * 128:(rh + 1) * 128],
                           g_t[:, ec * 512:(ec + 1) * 512],
                           start=(jc == 0), stop=(jc == 7))

            # mm2 trails mm1 by one j-block so the PE never waits on a GELU
            xg_prev = None
            for jc in range(8):
                w_ab = w_tiles[jc]

                # x[j-block jc] = h @ C1^T[:, jc]  (accumulate over d-chunks)
                xp = mm1p.tile([128, ROWS], DT)
                for dc in range(8):
                    mm(xp, w_ab[dc // 4][:, dc % 4, :], ht_q[dc // 2][:, dc % 2, :],
                       start=(dc == 0), stop=(dc == 7))

                xg = xpool.tile([128, ROWS], MM_DT)
                nc.scalar.activation(
                    out=xg, in_=xp,
                    func=mybir.ActivationFunctionType.Gelu,
                    bias=b1_t[:, jc:jc + 1], scale=sc_t[:, 0:1],
                )
                if xg_prev is not None:
                    mm2(jc - 1, xg_prev)
                xg_prev = xg
            mm2(7, xg_prev)

            # out = psum (+ b_out), then store one [128, 1024] tile per row half
            for rh in range(2):
                o_t = opool.tile([128, D], DT, name=f"o{rh}", tag=f"o{rh}")
                for ec in range(2):
                    sl = slice(ec * 512, (ec + 1) * 512)
                    if has_bout:
                        # (psum + S2*b_out) * (1/S2)
                        nc.vector.tensor_add(o_t[:, sl], out_ps[rh * 2 + ec],
                                             bout_t[:, sl])
                        nc.vector.tensor_scalar_mul(o_t[:, sl], o_t[:, sl],
                                                    sc_t[:, 1:2])
                    else:
                        nc.vector.tensor_scalar_mul(o_t[:, sl], out_ps[rh * 2 + ec],
                                                    sc_t[:, 1:2])
                nc.sync.dma_start(out=out[rh * 128:(rh + 1) * 128, :], in_=o_t)

    nc.compile()
    return nc


def _prepare_in_maps(h, prev_idx, fw_W1, fw_b1, fw_W2, fw_b2,
                     bw_W1, bw_b1, bw_W2, bw_b2,
                     Wv, fu_W1, fu_b1, fu_W2, fu_b2, Wo, chain_ratio):
    f8 = np.float64
    Wv64, fu_W164, fu_W264, Wo64 = (np.asarray(a, f8) for a in (Wv, fu_W1, fu_W2, Wo))
    W1a = fu_W164[:, :, :HD]
    W1bc = fu_W164[:, :, HD:2 * HD] + fu_W164[:, :, 2 * HD:]

    C1 = np.einsum("hec,hcd->hed", W1a, Wv64).reshape(D, D)
    S1 = 2.0 ** np.floor(np.log2(16.0 / max(np.abs(C1).max(), 1e-30)))
    C1T = np.ascontiguousarray((C1 * S1).T, np.float32)             # [d, j]
    # c1tw[jt, p, dc, j] = C1T[dc*128+p, jt*128+j]
    c1tw = np.ascontiguousarray(
        C1T.reshape(8, 128, 8, 128).transpose(2, 1, 0, 3))

    C3 = np.concatenate(
        [Wo64[:, i * HD:(i + 1) * HD] @ fu_W264[i] for i in range(H)], axis=1)
    S2 = 2.0 ** np.floor(np.log2(16.0 / max(np.abs(C3).max(), 1e-30)))
    G = np.ascontiguousarray((C3 * S2).T, np.float32)               # [j, e]

    hbar = np.asarray(h, f8).mean(axis=1)                           # [B, D]
    mf = np.einsum("hcd,bd->bhc", Wv64, hbar)
    bias1 = (np.asarray(fu_b1, f8)[None] + np.einsum("hec,bhc->bhe", W1bc, mf))
    # b1[b] laid out [p, jc] with j = jc*128 + p
    bias1 = np.ascontiguousarray(
        bias1.reshape(B, 8, 128).transpose(0, 2, 1), np.float32)

    # bout rides inside the S2-scaled PSUM, so pre-scale it
    bias_out = ((Wo64 @ np.asarray(fu_b2, f8).reshape(-1)) * S2).astype(np.float32)
    bout_row = np.ascontiguousarray(bias_out.reshape(1, D))
    scales_col = np.empty((128, 2), np.float32)
    scales_col[:, 0] = np.float32(1.0 / S1)
    scales_col[:, 1] = np.float32(1.0 / S2)

    h_flat = np.asarray(h, np.float32).reshape(B * N, D)
    in_maps = []
    for c in range(NCORES):
        rows = slice(c * ROWS, (c + 1) * ROWS)
        # ht[p, dc, r] = h_flat[row0+r, dc*128+p]
        ht_tiled = np.ascontiguousarray(
            h_flat[rows].T.reshape(8, 128, ROWS).transpose(1, 0, 2))
        in_maps.append({
            "ht": ht_tiled.astype(np.float16),
            "c1tw": c1tw.astype(np.float16),
            "g": G.astype(np.float16),
            "b1": bias1[c // (NCORES // B)],
            "scales": scales_col,
            "bout": bout_row,
        })
    return in_maps


def _small_outputs(prev_idx, chain_ratio):
    """Constant / index-select outputs (provably independent of the MLPs)."""
    sig = 1.0 / (1.0 + np.exp(-np.float64(np.asarray(chain_ratio))))
    thr = int(np.floor(sig * N))
    prev_clamped = np.clip(np.asarray(prev_idx), 0, N - 1).astype(np.int32)
    # uniform-softmax expectation of arange(N) is 511.5; the reference's
    # f32->int32 cast rounds to nearest on the device backend it runs under.
    uni = np.int32(round((N - 1) / 2))
    fwd = np.where(np.arange(N)[None, :] >= thr, prev_clamped, uni)
    bwd = np.full((B, N), uni, dtype=np.int32)
    strength = np.uint32(0x40FDCE6F).view(np.float32)   # 1 - log(1/N + 1e-8) as the
    avg = np.full((B, N), strength, dtype=np.float32)   # reference's backend computes it
    return fwd.astype(np.int32), bwd, avg


def _run(trace=False, **inputs):
    in_maps = _prepare_in_maps(**inputs)
    has_bout = bool(np.any(in_maps[0]["bout"]))
    key = ("nc", has_bout)
    if key not in _CACHE:
        _CACHE[key] = _build_nc(has_bout)
    nc = _CACHE[key]
    if not has_bout:
        for m in in_maps:
            del m["bout"]
    res = bass_utils.run_bass_kernel_spmd(
        nc, in_maps, core_ids=list(range(NCORES)), trace=trace)
    final = np.empty((B * N, D), np.float32)
    for c in range(NCORES):
        final[c * ROWS:(c + 1) * ROWS] = res.results[c]["out"]
    final = final.reshape(B, N, D)
    fwd, bwd, avg = _small_outputs(inputs["prev_idx"], inputs["chain_ratio"])
    return (final, fwd, bwd, avg), res


def kernel(**inputs):
    outs, _ = _run(trace=False, **inputs)
    return outs


def kernel_profiled(**inputs):
    outs, res = _run(trace=True, **inputs)
    return outs, res


# revision 22
# speedup vs baseline: 1.0389x; 1.0389x over previous
"""Trainium2 Bass kernel for nn_BiDirectionalMultiHeadPointer.

Math notes (derived from the reference, verified numerically):

The reference broadcasts a *scalar* per-(h,b,n) logit across the softmax
axis, so every softmax row is constant and the probabilities are exactly
uniform 1/N.  Consequences:

  * fwd/bwd encoder MLPs only feed those logits -> completely dead code.
  * attention-weighted features == mean over n of head_features, which is
    (mean_n h) @ Wv^T -- a rank-1-per-batch bias term after the fusion
    layer-1 weight is applied.
  * pointer targets before chaining are sum((1/N)*pos) = 511.5 -> int32
    (round-to-nearest on the device backend the reference runs under: 512).
  * entropy "strength" is the constant 1 - log(1/N + 1e-8).

With the per-head fusion weights folded into the value projection and the
output projection (exact linear algebra, done once on the host in f64):

  C1[j,d]   = sum_c fu_W1[h,e,:HD][e,c] * Wv[h,c,d]      (j = h*HD+e)
  b1[b,j]   = fu_b1[h,e] + (fu_W1[:,HD:2HD]+fu_W1[:,2HD:])[h,e,:] @ (Wv[h] @ mean_n h[b])
  G[j,e']   = sum_f Wo[e', h*HD+f] * fu_W2[h,f,e]        (j = h*HD+e)
  b_out     = Wo @ vec(fu_b2)

  final[b,n] = gelu(h[b,n] @ C1^T + b1[b]) @ G + b_out

The device kernel computes exactly that: two [256,1024]x[1024,1024]
matmuls with an erf-GELU between them, token-sharded 256 rows/core over
8 cores (a core's rows all belong to one batch, so b1 is a per-core
per-partition bias folded into the activation instruction).

Performance structure (from trace analysis):
  * matmul operands are fp16: the PE streams 2-byte dtypes at 1 cyc/row
    (fp32 needs 4, fp32r/TF32 2), fp16 carries the same 10-bit mantissa
    as the PE's TF32 mode, and DMA bytes halve. The tiny folded weights
    are pre-scaled by a power of two into fp16's normal range (exact);
    GELU's scale argument / a per-partition tensor_scalar undo it.
  * all DRAM operands are pre-tiled on the host so every DMA descriptor
    is a contiguous multi-KB run.
  * all large loads share the SP HWDGE ring, FIFO-ordered by consumption
    time (w0,w1,w2,g0,w3,g1,...); activations ride the gpsimd SWDGE
    queue in quarters so the first matmul starts ~4us earlier.
  * the second matmul trails the first by one j-block (software
    pipeline), so the PE never stalls on a GELU; accumulation groups for
    the 4 output PSUM banks stay open across the whole j loop.
  * a short burst of dense dummy matmuls on a memset tile warms the PE
    HAM clock gate (cold PE runs at 1.2 GHz for ~3.4us) while the first
    real operands are still in flight.
"""

import numpy as np

import concourse.mybir as mybir
import concourse.tile as tile
from concourse import bacc
from concourse import bass_utils

B, N, D, H, HD = 2, 1024, 1024, 16, 64
NCORES = 8
ROWS = (B * N) // NCORES      # 256 rows per core
DT = mybir.dt.float32
MM_DT = mybir.dt.float16      # matmul operand dtype: fp16 streams 1 cyc/row
# fp16 has the same 10-bit mantissa as the PE's fp32r/TF32 mode but half the
# bytes; the small-magnitude folded weights are pre-scaled by a power of two
# into fp16's normal range (exact), and the scale is undone on-chip (GELU's
# scale argument / a per-partition tensor_scalar multiply).

_CACHE = {}


def _build_nc(has_bout):
    """Build + compile the SPMD single-core program (same on all 8 cores)."""
    nc = bacc.Bacc(
        "TRN2",
        target_bir_lowering=False,
        debug=False,
        enable_asserts=False,
        num_devices=NCORES,
    )

    def mm(out, lhsT, rhs, start, stop):
        nc.tensor.matmul(out, lhsT, rhs, start=start, stop=stop)

    # host-pre-tiled so each [128, X] load is 128 contiguous 4-8KB runs
    hT = nc.dram_tensor("ht", [128, 8, ROWS], MM_DT, kind="ExternalInput").ap()
    c1tw = nc.dram_tensor("c1tw", [8, 128, 8, 128], MM_DT, kind="ExternalInput").ap()
    g = nc.dram_tensor("g", [D, D], MM_DT, kind="ExternalInput").ap()
    b1 = nc.dram_tensor("b1", [128, 8], DT, kind="ExternalInput").ap()
    scales = nc.dram_tensor("scales", [128, 2], DT, kind="ExternalInput").ap()
    bout = (nc.dram_tensor("bout", [1, D], DT, kind="ExternalInput").ap()
            if has_bout else None)
    out = nc.dram_tensor("out", [ROWS, D], DT, kind="ExternalOutput").ap()

    with tile.TileContext(nc) as tc:
        with (
            tc.tile_pool(name="hpool", bufs=1) as hpool,
            tc.tile_pool(name="wpool", bufs=6) as wpool,
            tc.tile_pool(name="gpool", bufs=8) as gpool,
            tc.tile_pool(name="xpool", bufs=8) as xpool,
            tc.tile_pool(name="bpool", bufs=1) as bpool,
            tc.tile_pool(name="opool", bufs=4) as opool,
            tc.tile_pool(name="mm1p", bufs=2, space="PSUM") as mm1p,
            tc.tile_pool(name="mm2p", bufs=1, space="PSUM") as mm2p,
        ):
            # --- resident inputs ----------------------------------------------
            # ht in quarters on the gpsimd (SWDGE) queue so the first
            # matmul isn't queued behind it on the weight ring
            ht_q = []
            b1_t = None
            for hh in range(4):
                ht_t = hpool.tile([128, 2, ROWS], MM_DT, name=f"ht{hh}",
                                  tag=f"ht{hh}")   # [p, dc, r], d = (2*hh+dc)*128+p
                nc.gpsimd.dma_start(out=ht_t, in_=hT[:, 2 * hh:2 * (hh + 1), :])
                ht_q.append(ht_t)
                if hh == 0:
                    b1_t = bpool.tile([128, 8], DT)        # j = jc*128 + p
                    nc.gpsimd.dma_start(out=b1_t, in_=b1)
                    sc_t = bpool.tile([128, 2], DT)        # [1/S1, 1/S2]
                    nc.gpsimd.dma_start(out=sc_t, in_=scales)

            if has_bout:
                bout_t = bpool.tile([128, D], DT)
                nc.gpsimd.dma_start(out=bout_t, in_=bout.broadcast_to((128, D)))

            out_ps = [mm2p.tile([128, 512], DT, name=f"outp{i}", tag=f"outp{i}")
                      for i in range(4)]

            # HAM warm-up: dense N=512 matmuls on a memset tile keep the PE
            # busy-window >90% occupied until the first real matmul, so the
            # clock gate opens (cold PE = 1.2 GHz) before real work arrives
            one_c = nc.const_aps.aps[(mybir.dt.float32, 1.0)]
            wsrc = bpool.tile([128, 512], MM_DT, name="wsrc", tag="wsrc")
            nc.vector.memset(wsrc, 0.0)
            one16 = bpool.tile([128, 1], MM_DT, name="one16", tag="one16")
            nc.vector.tensor_copy(one16, one_c)
            # preload the Gelu PWP table while DMAs stream, so the first
            # real GELU doesn't pay the ~1.3us ACT_TABLE_LOAD mid-pipeline
            gscr = bpool.tile([128, 1], DT, name="gscr", tag="gscr")
            nc.scalar.activation(out=gscr, in_=one_c,
                                 func=mybir.ActivationFunctionType.Gelu,
                                 bias=0.0, scale=1.0)
            warm = mm1p.tile([1, 512], DT, name="warm", tag="warm")
            for i in range(10):
                nc.tensor.matmul(warm, one16, wsrc,
                                 start=(i == 0), stop=(i == 9))

            # ---- all large loads on the sync HWDGE ring, FIFO-ordered by
            # consumption time: w blocks pace mm1; g[jc] is needed one jc
            # later by mm2, so it rides two slots behind its w ------------
            w_tiles, g_tiles = {}, {}

            def load_w(jc):
                if jc < 2:
                    w_ab = []
                    for hh in range(2):
                        w_t = wpool.tile([128, 4, 128], MM_DT, name=f"w{hh}",
                                         tag=f"w{hh}")
                        nc.sync.dma_start(
                            out=w_t, in_=c1tw[jc, :, 4 * hh:4 * (hh + 1), :])
                        w_ab.append(w_t)
                    w_tiles[jc] = w_ab
                else:
                    w_t = wpool.tile([128, 8, 128], MM_DT, name="wf", tag="wf")
                    nc.sync.dma_start(out=w_t, in_=c1tw[jc])
                    w_tiles[jc] = [w_t[:, :4, :], w_t[:, 4:, :]]

            def load_g(jc):
                g_t = gpool.tile([128, D], MM_DT, name="gt", tag="gt")
                nc.sync.dma_start(out=g_t, in_=g[jc * 128:(jc + 1) * 128, :])
                g_tiles[jc] = g_t

            for item in ("w0", "w1", "w2", "g0", "w3", "g1", "w4", "g2",
                         "w5", "g3", "w6", "g4", "w7", "g5", "g6", "g7"):
                (load_w if item[0] == "w" else load_g)(int(item[1]))

            def mm2(jc, xg):
                # out[rh, ec] += xg^T[rh] @ G[jc, ec]
                g_t = g_tiles[jc]
                for rh in range(2):
                    for ec in range(2):
                        mm(out_ps[rh * 2 + ec],
                           xg[:, rh * 128:(rh + 1) * 128],
                           g_t[:, ec * 512:(ec + 1) * 512],
                           start=(jc == 0), stop=(jc == 7))

            # mm2 trails mm1 by one j-block so the PE never waits on a GELU
            xg_prev = None
            for jc in range(8):
                w_ab = w_tiles[jc]

                # x[j-block jc] = h @ C1^T[:, jc]  (accumulate over d-chunks)
                xp = mm1p.tile([128, ROWS], DT)
                for dc in range(8):
                    mm(xp, w_ab[dc // 4][:, dc % 4, :], ht_q[dc // 2][:, dc % 2, :],
                       start=(dc == 0), stop=(dc == 7))

                xg = xpool.tile([128, ROWS], MM_DT)
                nc.scalar.activation(
                    out=xg, in_=xp,
                    func=mybir.ActivationFunctionType.Gelu,
                    bias=b1_t[:, jc:jc + 1], scale=sc_t[:, 0:1],
                )
                if xg_prev is not None:
                    mm2(jc - 1, xg_prev)
                xg_prev = xg
            mm2(7, xg_prev)

            # out = psum (+ b_out), then store one [128, 1024] tile per row half
            for rh in range(2):
                o_t = opool.tile([128, D], DT, name=f"o{rh}", tag=f"o{rh}")
                for ec in range(2):
                    sl = slice(ec * 512, (ec + 1) * 512)
                    if has_bout:
                        # (psum + S2*b_out) * (1/S2)
                        nc.vector.tensor_add(o_t[:, sl], out_ps[rh * 2 + ec],
                                             bout_t[:, sl])
                        nc.vector.tensor_scalar_mul(o_t[:, sl], o_t[:, sl],
                                                    sc_t[:, 1:2])
                    else:
                        nc.vector.tensor_scalar_mul(o_t[:, sl], out_ps[rh * 2 + ec],
                                                    sc_t[:, 1:2])
                nc.sync.dma_start(out=out[rh * 128:(rh + 1) * 128, :], in_=o_t)

    nc.compile()
    return nc


def _prepare_in_maps(h, prev_idx, fw_W1, fw_b1, fw_W2, fw_b2,
                     bw_W1, bw_b1, bw_W2, bw_b2,
                     Wv, fu_W1, fu_b1, fu_W2, fu_b2, Wo, chain_ratio):
    f8 = np.float64
    Wv64, fu_W164, fu_W264, Wo64 = (np.asarray(a, f8) for a in (Wv, fu_W1, fu_W2, Wo))
    W1a = fu_W164[:, :, :HD]
    W1bc = fu_W164[:, :, HD:2 * HD] + fu_W164[:, :, 2 * HD:]

    C1 = np.einsum("hec,hcd->hed", W1a, Wv64).reshape(D, D)
    S1 = 2.0 ** np.floor(np.log2(16.0 / max(np.abs(C1).max(), 1e-30)))
    C1T = np.ascontiguousarray((C1 * S1).T, np.float32)             # [d, j]
    # c1tw[jt, p, dc, j] = C1T[dc*128+p, jt*128+j]
    c1tw = np.ascontiguousarray(
        C1T.reshape(8, 128, 8, 128).transpose(2, 1, 0, 3))

    C3 = np.concatenate(
        [Wo64[:, i * HD:(i + 1) * HD] @ fu_W264[i] for i in range(H)], axis=1)
    S2 = 2.0 ** np.floor(np.log2(16.0 / max(np.abs(C3).max(), 1e-30)))
    G = np.ascontiguousarray((C3 * S2).T, np.float32)               # [j, e]

    hbar = np.asarray(h, f8).mean(axis=1)                           # [B, D]
    mf = np.einsum("hcd,bd->bhc", Wv64, hbar)
    bias1 = (np.asarray(fu_b1, f8)[None] + np.einsum("hec,bhc->bhe", W1bc, mf))
    # b1[b] laid out [p, jc] with j = jc*128 + p
    bias1 = np.ascontiguousarray(
        bias1.reshape(B, 8, 128).transpose(0, 2, 1), np.float32)

    # bout rides inside the S2-scaled PSUM, so pre-scale it
    bias_out = ((Wo64 @ np.asarray(fu_b2, f8).reshape(-1)) * S2).astype(np.float32)
    bout_row = np.ascontiguousarray(bias_out.reshape(1, D))
    scales_col = np.empty((128, 2), np.float32)
    scales_col[:, 0] = np.float32(1.0 / S1)
    scales_col[:, 1] = np.float32(1.0 / S2)

    h_flat = np.asarray(h, np.float32).reshape(B * N, D)
    in_maps = []
    for c in range(NCORES):
        rows = slice(c * ROWS, (c + 1) * ROWS)
        # ht[p, dc, r] = h_flat[row0+r, dc*128+p]
        ht_tiled = np.ascontiguousarray(
            h_flat[rows].T.reshape(8, 128, ROWS).transpose(1, 0, 2))
        in_maps.append({
            "ht": ht_tiled.astype(np.float16),
            "c1tw": c1tw.astype(np.float16),
            "g": G.astype(np.float16),
            "b1": bias1[c // (NCORES // B)],
            "scales": scales_col,
            "bout": bout_row,
        })
    return in_maps


def _small_outputs(prev_idx, chain_ratio):
    """Constant / index-select outputs (provably independent of the MLPs)."""
    sig = 1.0 / (1.0 + np.exp(-np.float64(np.asarray(chain_ratio))))
    thr = int(np.floor(sig * N))
    prev_clamped = np.clip(np.asarray(prev_idx), 0, N - 1).astype(np.int32)
    # uniform-softmax expectation of arange(N) is 511.5; the reference's
    # f32->int32 cast rounds to nearest on the device backend it runs under.
    uni = np.int32(round((N - 1) / 2))
    fwd = np.where(np.arange(N)[None, :] >= thr, prev_clamped, uni)
    bwd = np.full((B, N), uni, dtype=np.int32)
    strength = np.uint32(0x40FDCE6F).view(np.float32)   # 1 - log(1/N + 1e-8) as the
    avg = np.full((B, N), strength, dtype=np.float32)   # reference's backend computes it
    return fwd.astype(np.int32), bwd, avg


def _run(trace=False, **inputs):
    in_maps = _prepare_in_maps(**inputs)
    has_bout = bool(np.any(in_maps[0]["bout"]))
    key = ("nc", has_bout)
    if key not in _CACHE:
        _CACHE[key] = _build_nc(has_bout)
    nc = _CACHE[key]
    if not has_bout:
        for m in in_maps:
            del m["bout"]
    res = bass_utils.run_bass_kernel_spmd(
        nc, in_maps, core_ids=list(range(NCORES)), trace=trace)
    final = np.empty((B * N, D), np.float32)
    for c in range(NCORES):
        final[c * ROWS:(c + 1) * ROWS] = res.results[c]["out"]
    final = final.reshape(B, N, D)
    fwd, bwd, avg = _small_outputs(inputs["prev_idx"], inputs["chain_ratio"])
    return (final, fwd, bwd, avg), res


def kernel(**inputs):
    outs, _ = _run(trace=False, **inputs)
    return outs


def kernel_profiled(**inputs):
    outs, res = _run(trace=True, **inputs)
    return outs, res


# revision 23
# speedup vs baseline: 1.1109x; 1.0694x over previous
"""Trainium2 Bass kernel for nn_BiDirectionalMultiHeadPointer.

Math notes (derived from the reference, verified numerically):

The reference broadcasts a *scalar* per-(h,b,n) logit across the softmax
axis, so every softmax row is constant and the probabilities are exactly
uniform 1/N.  Consequences:

  * fwd/bwd encoder MLPs only feed those logits -> completely dead code.
  * attention-weighted features == mean over n of head_features, which is
    (mean_n h) @ Wv^T -- a rank-1-per-batch bias term after the fusion
    layer-1 weight is applied.
  * pointer targets before chaining are sum((1/N)*pos) = 511.5 -> int32
    (round-to-nearest on the device backend the reference runs under: 512).
  * entropy "strength" is the constant 1 - log(1/N + 1e-8).

With the per-head fusion weights folded into the value projection and the
output projection (exact linear algebra, done once on the host in f64):

  C1[j,d]   = sum_c fu_W1[h,e,:HD][e,c] * Wv[h,c,d]      (j = h*HD+e)
  b1[b,j]   = fu_b1[h,e] + (fu_W1[:,HD:2HD]+fu_W1[:,2HD:])[h,e,:] @ (Wv[h] @ mean_n h[b])
  G[j,e']   = sum_f Wo[e', h*HD+f] * fu_W2[h,f,e]        (j = h*HD+e)
  b_out     = Wo @ vec(fu_b2)

  final[b,n] = gelu(h[b,n] @ C1^T + b1[b]) @ G + b_out

The device kernel computes exactly that: two [256,1024]x[1024,1024]
matmuls with an erf-GELU between them, token-sharded 256 rows/core over
8 cores (a core's rows all belong to one batch, so b1 is a per-core
per-partition bias folded into the activation instruction).

Performance structure (from trace analysis):
  * matmul operands are fp16: the PE streams 2-byte dtypes at 1 cyc/row
    (fp32 needs 4, fp32r/TF32 2), fp16 carries the same 10-bit mantissa
    as the PE's TF32 mode, and DMA bytes halve. The tiny folded weights
    are pre-scaled by a power of two into fp16's normal range (exact);
    GELU's scale argument / a per-partition tensor_scalar undo it.
  * all DRAM operands are pre-tiled on the host so every DMA descriptor
    is a contiguous multi-KB run.
  * all large loads share the SP HWDGE ring, FIFO-ordered by consumption
    time (w0,w1,w2,g0,w3,g1,...); activations ride the gpsimd SWDGE
    queue in quarters so the first matmul starts ~4us earlier.
  * the second matmul trails the first by one j-block (software
    pipeline), so the PE never stalls on a GELU; accumulation groups for
    the 4 output PSUM banks stay open across the whole j loop.
  * a short burst of dense dummy matmuls on a memset tile warms the PE
    HAM clock gate (cold PE runs at 1.2 GHz for ~3.4us) while the first
    real operands are still in flight.
"""

import numpy as np

import concourse.mybir as mybir
import concourse.tile as tile
from concourse import bacc
from concourse import bass_utils

B, N, D, H, HD = 2, 1024, 1024, 16, 64
NCORES = 8
ROWS = (B * N) // NCORES      # 256 rows per core
DT = mybir.dt.float32
MM_DT = mybir.dt.float16      # matmul operand dtype: fp16 streams 1 cyc/row
# fp16 has the same 10-bit mantissa as the PE's fp32r/TF32 mode but half the
# bytes; the small-magnitude folded weights are pre-scaled by a power of two
# into fp16's normal range (exact), and the scale is undone on-chip (GELU's
# scale argument / a per-partition tensor_scalar multiply).

_CACHE = {}


def _build_nc(has_bout):
    """Build + compile the SPMD single-core program (same on all 8 cores)."""
    nc = bacc.Bacc(
        "TRN2",
        target_bir_lowering=False,
        debug=False,
        enable_asserts=False,
        num_devices=NCORES,
    )

    def mm(out, lhsT, rhs, start, stop):
        nc.tensor.matmul(out, lhsT, rhs, start=start, stop=stop)

    # host-pre-tiled so each [128, X] load is 128 contiguous 4-8KB runs
    hT = nc.dram_tensor("ht", [128, 8, ROWS], MM_DT, kind="ExternalInput").ap()
    c1tw = nc.dram_tensor("c1tw", [8, 128, 8, 128], MM_DT, kind="ExternalInput").ap()
    g = nc.dram_tensor("g", [D, D], MM_DT, kind="ExternalInput").ap()
    b1 = nc.dram_tensor("b1", [128, 8], DT, kind="ExternalInput").ap()
    scales = nc.dram_tensor("scales", [128, 2], DT, kind="ExternalInput").ap()
    bout = (nc.dram_tensor("bout", [1, D], DT, kind="ExternalInput").ap()
            if has_bout else None)
    out = nc.dram_tensor("out", [ROWS, D], DT, kind="ExternalOutput").ap()

    with tile.TileContext(nc) as tc:
        with (
            tc.tile_pool(name="hpool", bufs=1) as hpool,
            tc.tile_pool(name="wpool", bufs=6) as wpool,
            tc.tile_pool(name="gpool", bufs=8) as gpool,
            tc.tile_pool(name="xpool", bufs=8) as xpool,
            tc.tile_pool(name="bpool", bufs=1) as bpool,
            tc.tile_pool(name="opool", bufs=4) as opool,
            tc.tile_pool(name="mm1p", bufs=2, space="PSUM") as mm1p,
            tc.tile_pool(name="mm2p", bufs=1, space="PSUM") as mm2p,
        ):
            # --- resident inputs ----------------------------------------------
            # ht in quarters on the gpsimd (SWDGE) queue so the first
            # matmul isn't queued behind it on the weight ring
            ht_q = []
            b1_t = bpool.tile([128, 8], DT)                # j = jc*128 + p
            nc.gpsimd.dma_start(out=b1_t, in_=b1)
            sc_t = bpool.tile([128, 2], DT)                # [1/S1, 1/S2]
            nc.gpsimd.dma_start(out=sc_t, in_=scales)

            def load_ht(hh):
                ht_t = hpool.tile([128, 2, ROWS], MM_DT, name=f"ht{hh}",
                                  tag=f"ht{hh}")   # [p, dc, r], d = (2*hh+dc)*128+p
                nc.sync.dma_start(out=ht_t, in_=hT[:, 2 * hh:2 * (hh + 1), :])
                ht_q.append(ht_t)

            if has_bout:
                bout_t = bpool.tile([128, D], DT)
                nc.gpsimd.dma_start(out=bout_t, in_=bout.broadcast_to((128, D)))

            out_ps = [mm2p.tile([128, 512], DT, name=f"outp{i}", tag=f"outp{i}")
                      for i in range(4)]

            # HAM warm-up: dense N=512 matmuls on a memset tile keep the PE
            # busy-window >90% occupied until the first real matmul, so the
            # clock gate opens (cold PE = 1.2 GHz) before real work arrives
            one_c = nc.const_aps.aps[(mybir.dt.float32, 1.0)]
            wsrc = bpool.tile([128, 512], MM_DT, name="wsrc", tag="wsrc")
            nc.vector.memset(wsrc, 0.0)
            one16 = bpool.tile([128, 1], MM_DT, name="one16", tag="one16")
            nc.vector.tensor_copy(one16, one_c)
            # preload the Gelu PWP table while DMAs stream, so the first
            # real GELU doesn't pay the ~1.3us ACT_TABLE_LOAD mid-pipeline
            gscr = bpool.tile([128, 1], DT, name="gscr", tag="gscr")
            nc.scalar.activation(out=gscr, in_=one_c,
                                 func=mybir.ActivationFunctionType.Gelu,
                                 bias=0.0, scale=1.0)
            warm = mm1p.tile([1, 512], DT, name="warm", tag="warm")
            for i in range(10):
                nc.tensor.matmul(warm, one16, wsrc,
                                 start=(i == 0), stop=(i == 9))

            # ---- all large loads on the sync HWDGE ring, FIFO-ordered by
            # consumption time: w blocks pace mm1; g[jc] is needed one jc
            # later by mm2, so it rides two slots behind its w ------------
            w_tiles, g_tiles = {}, {}

            def load_w(jc):
                if jc < 2:
                    w_ab = []
                    for hh in range(2):
                        w_t = wpool.tile([128, 4, 128], MM_DT, name=f"w{hh}",
                                         tag=f"w{hh}")
                        nc.sync.dma_start(
                            out=w_t, in_=c1tw[jc, :, 4 * hh:4 * (hh + 1), :])
                        w_ab.append(w_t)
                    w_tiles[jc] = w_ab
                else:
                    w_t = wpool.tile([128, 8, 128], MM_DT, name="wf", tag="wf")
                    nc.sync.dma_start(out=w_t, in_=c1tw[jc])
                    w_tiles[jc] = [w_t[:, :4, :], w_t[:, 4:, :]]

            def load_g(jc):
                g_t = gpool.tile([128, D], MM_DT, name="gt", tag="gt")
                nc.sync.dma_start(out=g_t, in_=g[jc * 128:(jc + 1) * 128, :])
                g_tiles[jc] = g_t

            for item in ("h0", "w0", "h1", "w1", "h2", "h3", "w2", "g0",
                         "w3", "g1", "w4", "g2", "w5", "g3", "w6", "g4",
                         "w7", "g5", "g6", "g7"):
                {"h": load_ht, "w": load_w, "g": load_g}[item[0]](int(item[1]))

            def mm2(jc, xg):
                # out[rh, ec] += xg^T[rh] @ G[jc, ec]
                g_t = g_tiles[jc]
                for rh in range(2):
                    for ec in range(2):
                        mm(out_ps[rh * 2 + ec],
                           xg[:, rh * 128:(rh + 1) * 128],
                           g_t[:, ec * 512:(ec + 1) * 512],
                           start=(jc == 0), stop=(jc == 7))

            # mm2 trails mm1 by one j-block so the PE never waits on a GELU
            xg_prev = None
            for jc in range(8):
                w_ab = w_tiles[jc]

                # x[j-block jc] = h @ C1^T[:, jc]  (accumulate over d-chunks)
                xp = mm1p.tile([128, ROWS], DT)
                for dc in range(8):
                    mm(xp, w_ab[dc // 4][:, dc % 4, :], ht_q[dc // 2][:, dc % 2, :],
                       start=(dc == 0), stop=(dc == 7))

                xg = xpool.tile([128, ROWS], MM_DT)
                nc.scalar.activation(
                    out=xg, in_=xp,
                    func=mybir.ActivationFunctionType.Gelu,
                    bias=b1_t[:, jc:jc + 1], scale=sc_t[:, 0:1],
                )
                if xg_prev is not None:
                    mm2(jc - 1, xg_prev)
                xg_prev = xg
            mm2(7, xg_prev)

            # out = psum (+ b_out), each [128,512] half stored as soon as
            # its unscale completes
            for rh in range(2):
                for ec in range(2):
                    sl = slice(ec * 512, (ec + 1) * 512)
                    o_t = opool.tile([128, 512], DT, name=f"o{rh}{ec}",
                                     tag=f"o{rh}{ec}")
                    if has_bout:
                        # (psum + S2*b_out) * (1/S2)
                        nc.vector.tensor_add(o_t, out_ps[rh * 2 + ec],
                                             bout_t[:, sl])
                        nc.vector.tensor_scalar_mul(o_t, o_t, sc_t[:, 1:2])
                    else:
                        nc.vector.tensor_scalar_mul(o_t, out_ps[rh * 2 + ec],
                                                    sc_t[:, 1:2])
                    nc.sync.dma_start(
                        out=out[rh * 128:(rh + 1) * 128, sl], in_=o_t)

    nc.compile()
    return nc


def _prepare_in_maps(h, prev_idx, fw_W1, fw_b1, fw_W2, fw_b2,
                     bw_W1, bw_b1, bw_W2, bw_b2,
                     Wv, fu_W1, fu_b1, fu_W2, fu_b2, Wo, chain_ratio):
    f8 = np.float64
    Wv64, fu_W164, fu_W264, Wo64 = (np.asarray(a, f8) for a in (Wv, fu_W1, fu_W2, Wo))
    W1a = fu_W164[:, :, :HD]
    W1bc = fu_W164[:, :, HD:2 * HD] + fu_W164[:, :, 2 * HD:]

    C1 = np.einsum("hec,hcd->hed", W1a, Wv64).reshape(D, D)
    S1 = 2.0 ** np.floor(np.log2(16.0 / max(np.abs(C1).max(), 1e-30)))
    C1T = np.ascontiguousarray((C1 * S1).T, np.float32)             # [d, j]
    # c1tw[jt, p, dc, j] = C1T[dc*128+p, jt*128+j]
    c1tw = np.ascontiguousarray(
        C1T.reshape(8, 128, 8, 128).transpose(2, 1, 0, 3))

    C3 = np.concatenate(
        [Wo64[:, i * HD:(i + 1) * HD] @ fu_W264[i] for i in range(H)], axis=1)
    S2 = 2.0 ** np.floor(np.log2(16.0 / max(np.abs(C3).max(), 1e-30)))
    G = np.ascontiguousarray((C3 * S2).T, np.float32)               # [j, e]

    hbar = np.asarray(h, f8).mean(axis=1)                           # [B, D]
    mf = np.einsum("hcd,bd->bhc", Wv64, hbar)
    bias1 = (np.asarray(fu_b1, f8)[None] + np.einsum("hec,bhc->bhe", W1bc, mf))
    # b1[b] laid out [p, jc] with j = jc*128 + p
    bias1 = np.ascontiguousarray(
        bias1.reshape(B, 8, 128).transpose(0, 2, 1), np.float32)

    # bout rides inside the S2-scaled PSUM, so pre-scale it
    bias_out = ((Wo64 @ np.asarray(fu_b2, f8).reshape(-1)) * S2).astype(np.float32)
    bout_row = np.ascontiguousarray(bias_out.reshape(1, D))
    scales_col = np.empty((128, 2), np.float32)
    scales_col[:, 0] = np.float32(1.0 / S1)
    scales_col[:, 1] = np.float32(1.0 / S2)

    h_flat = np.asarray(h, np.float32).reshape(B * N, D)
    in_maps = []
    for c in range(NCORES):
        rows = slice(c * ROWS, (c + 1) * ROWS)
        # ht[p, dc, r] = h_flat[row0+r, dc*128+p]
        ht_tiled = np.ascontiguousarray(
            h_flat[rows].T.reshape(8, 128, ROWS).transpose(1, 0, 2))
        in_maps.append({
            "ht": ht_tiled.astype(np.float16),
            "c1tw": c1tw.astype(np.float16),
            "g": G.astype(np.float16),
            "b1": bias1[c // (NCORES // B)],
            "scales": scales_col,
            "bout": bout_row,
        })
    return in_maps


def _small_outputs(prev_idx, chain_ratio):
    """Constant / index-select outputs (provably independent of the MLPs)."""
    sig = 1.0 / (1.0 + np.exp(-np.float64(np.asarray(chain_ratio))))
    thr = int(np.floor(sig * N))
    prev_clamped = np.clip(np.asarray(prev_idx), 0, N - 1).astype(np.int32)
    # uniform-softmax expectation of arange(N) is 511.5; the reference's
    # f32->int32 cast rounds to nearest on the device backend it runs under.
    uni = np.int32(round((N - 1) / 2))
    fwd = np.where(np.arange(N)[None, :] >= thr, prev_clamped, uni)
    bwd = np.full((B, N), uni, dtype=np.int32)
    strength = np.uint32(0x40FDCE6F).view(np.float32)   # 1 - log(1/N + 1e-8) as the
    avg = np.full((B, N), strength, dtype=np.float32)   # reference's backend computes it
    return fwd.astype(np.int32), bwd, avg


def _run(trace=False, **inputs):
    in_maps = _prepare_in_maps(**inputs)
    has_bout = bool(np.any(in_maps[0]["bout"]))
    key = ("nc", has_bout)
    if key not in _CACHE:
        _CACHE[key] = _build_nc(has_bout)
    nc = _CACHE[key]
    if not has_bout:
        for m in in_maps:
            del m["bout"]
    res = bass_utils.run_bass_kernel_spmd(
        nc, in_maps, core_ids=list(range(NCORES)), trace=trace)
    final = np.empty((B * N, D), np.float32)
    for c in range(NCORES):
        final[c * ROWS:(c + 1) * ROWS] = res.results[c]["out"]
    final = final.reshape(B, N, D)
    fwd, bwd, avg = _small_outputs(inputs["prev_idx"], inputs["chain_ratio"])
    return (final, fwd, bwd, avg), res


def kernel(**inputs):
    outs, _ = _run(trace=False, **inputs)
    return outs


def kernel_profiled(**inputs):
    outs, res = _run(trace=True, **inputs)
    return outs, res
